# revision 16
# baseline (speedup 1.0000x reference)
"""Category-specific linear on 8 trn2 cores — hidden-dim sharding, resident W.

v3 changes vs v2 (486us baseline):
- Output in bf16 (halves out DMA 67->33.5MB/core; total HBM 151->117MB
  removes the ~90% HBM saturation that caused mid-kernel W-prefetch stalls).
  Host upcasts to fp32.
- Warmup matmuls are dep-free (no memzero) and issued first, so the PE
  starts at ~0.3us instead of ~9us and the HAM clock-gate flips during the
  unavoidable first-x DMA wait.
- First x pair loaded as two per-sample DMAs interleaved with the first
  group's per-ko W pieces, so the k-pipeline of sample 0 never starves.
- W prefetch lookahead 3 groups (was 2), each as two half-tile DMAs so
  first-half matmuls can start before the full megabyte lands.
- Last sample drains PSUM per-m (4 small copies+DMAs) to shorten the tail.
"""

import numpy as np
import ml_dtypes

B = 64
S = 512
DIN = 1024
DH = 4096
C = 16
NCORES = 8
NSH = DH // NCORES   # 512
P = 128
KO = DIN // P        # 8
MO = S // P          # 4

LAST_RESULTS = None


def _build_program(cats):
    import concourse.bacc as bacc
    import concourse.mybir as mybir
    import concourse.tile as tile

    nc = bacc.Bacc("TRN2", target_bir_lowering=False)

    assert B % 2 == 0
    x_d = nc.dram_tensor("x", (B // 2, P, 2, KO, S), mybir.dt.bfloat16,
                         kind="ExternalInput")
    w_d = nc.dram_tensor("w", (C, P, KO, NSH), mybir.dt.bfloat16,
                         kind="ExternalInput")
    out_d = nc.dram_tensor("out", (B, P, MO, NSH), mybir.dt.bfloat16,
                           kind="ExternalOutput")

    # Category-grouped processing order: samples of the same category are
    # consecutive, so each category's W slice is prefetched once and covered
    # by ~n_c * 6.9us of compute. proc_order[i] = original sample index.
    used = []
    for c in cats:
        if c not in used:
            used.append(c)
    proc_order = [j for c in used for j in range(B) if cats[j] == c]
    # group g -> category used[g]; first processed position of each group
    group_start = {}
    for i, j in enumerate(proc_order):
        group_start.setdefault(cats[j], i)
    n_groups = len(used)

    with tile.TileContext(nc) as tc:
        with (
            tc.tile_pool(name="wpool", bufs=1) as wp,
            tc.tile_pool(name="xpool", bufs=3) as xb,
            tc.tile_pool(name="opool", bufs=3) as ob,
            tc.tile_pool(name="psum", bufs=2, space="PSUM") as ps,
        ):
            w_tiles = {}

            def emit_w(g):
                if g >= n_groups:
                    return
                c = used[g]
                if c in w_tiles:
                    return
                t = wp.tile([P, KO, NSH], mybir.dt.bfloat16, tag=f"w{c}")
                # two half-tile DMAs: matmuls on ko<4 only wait for half 0
                nc.sync.dma_start(t[:, 0:KO // 2, :], w_d[c, :, 0:KO // 2, :])
                nc.sync.dma_start(t[:, KO // 2:KO, :], w_d[c, :, KO // 2:KO, :])
                w_tiles[c] = t

            # PE warmup: matmuls on DVE-memset scratch, gated only on the two
            # fast memsets (~0.5us) rather than any DMA. They run during the
            # initial DMA wait so the HAM clock-gate reaches 8/8 before real
            # data lands.
            warm_l = wp.tile([P, P], mybir.dt.bfloat16, tag="warm_l")
            warm_r = wp.tile([P, NSH], mybir.dt.bfloat16, tag="warm_r")
            nc.vector.memset(warm_l[:], 0.0)
            nc.vector.memset(warm_r[:], 0.0)
            warm_p = ps.tile([P, MO, NSH], mybir.dt.float32, tag="ps")
            # 16 warmups = ~6.8us of cold-rate PE busy: enough for the HAM
            # clock-gate to flip to 8/8 (~3.4us sustained) AND to bridge until
            # the first sample's x/W DMAs have fully landed (~12us), so real
            # matmuls start warm with zero data stutter.
            for _ in range(16):
                nc.tensor.matmul(warm_p[:, 0, :], warm_l[:], warm_r[:],
                                 start=True, stop=True)

            # Preamble DMAs in DEADLINE order. Each sync.dma_start costs
            # ~0.65us of issue time on the sync sequencer and the transfers
            # queue up behind each other at ~2.8us/MiB, so issue order must
            # match the order the data is consumed: otherwise the second x
            # pair queues behind 4MB of W prefetch and the PE stalls ~5us at
            # t~14us (observed in v3), re-throttling the HAM clock gate.
            c0 = used[0]
            w0k = [None] * KO
            xt_pairs = {}

            def emit_xpair(p):
                t = xb.tile([P, 2, KO, S], mybir.dt.bfloat16, tag="x")
                nc.sync.dma_start(t[:], x_d[p])
                xt_pairs[p] = t

            def emit_w0(ks):
                for k in ks:
                    twk = wp.tile([P, NSH], mybir.dt.bfloat16, tag=f"w{c0}_k{k}")
                    nc.sync.dma_start(twk[:], w_d[c0, :, k, :])
                    w0k[k] = twk

            # Only pair 1 is pre-issued beyond pair 0: DMA-completion waits are
            # thresholds on shared queue counters, so pre-queueing transfers
            # that are needed LATER inflates the wait of everything behind
            # them in the queue (measured: a 6-pair preamble stalled the PE
            # ~7us at t~59us waiting for a counter that included xp4/xp5).
            T_MM = 6.9  # us of matmul per sample
            PRE_PAIRS = min(2, B // 2)
            xt0 = xb.tile([P, 2, KO, S], mybir.dt.bfloat16, tag="x")
            xt_pairs[0] = xt0
            H = KO // 2
            pre = [(-3.0, "xs0a"), (-2.0, "w0k03"), (-1.0, "xs0b"),
                   (0.0, "w0k47"), (1.0, "xs1")]
            for p in range(1, PRE_PAIRS):
                pre.append((4 + 2 * T_MM * p - 6.0, f"xp{p}"))
            for g in range(1, min(4, n_groups)):
                pre.append((4 + T_MM * group_start[used[g]] - 8.0, f"wg{g}"))
            pre.sort()
            for _, what in pre:
                if what == "xs0a":
                    nc.sync.dma_start(xt0[:, 0, 0:H], x_d[0, :, 0, 0:H])
                elif what == "xs0b":
                    nc.sync.dma_start(xt0[:, 0, H:KO], x_d[0, :, 0, H:KO])
                elif what == "xs1":
                    nc.sync.dma_start(xt0[:, 1], x_d[0, :, 1])
                elif what == "w0k03":
                    emit_w0(range(0, 4))
                elif what == "w0k47":
                    emit_w0(range(4, KO))
                elif what.startswith("xp"):
                    emit_xpair(int(what[2:]))
                else:
                    emit_w(int(what[2:]))

            for i in range(B):
                j = proc_order[i]
                c = cats[j]
                g = used.index(c)
                if group_start[c] == i and g + 3 < n_groups:
                    emit_w(g + 3)
                if i % 2 == 0:
                    p = i // 2
                    if p in xt_pairs:
                        xt = xt_pairs.pop(p)
                    else:
                        xt = xb.tile([P, 2, KO, S], mybir.dt.bfloat16, tag="x")
                        nc.sync.dma_start(xt[:], x_d[p])
                pt = ps.tile([P, MO, NSH], mybir.dt.float32, tag="ps")
                for m in range(MO):
                    for k in range(KO):
                        rhs = w0k[k][:] if c == c0 else w_tiles[c][:, k, :]
                        nc.tensor.matmul(
                            pt[:, m, :],
                            xt[:, i % 2, k, m * P:(m + 1) * P],
                            rhs,
                            start=(k == 0),
                            stop=(k == KO - 1),
                        )
                # out DMAs are issued from GpSimd, NOT sync: DMA-completion
                # waits are thresholds on a per-queue counter, so if out
                # writes share a queue with x/W reads, every later x consumer
                # transitively waits on cast-paced out completions (measured
                # as a one-time ~7us PE stall at the first in-loop x pair).
                if i == B - 1:
                    # per-m drain: copy+store each 128x512 slab as soon as its
                    # k-accumulation finishes, shortening the kernel tail.
                    # These last stores go back on sync: no x/W reads remain
                    # after them (no counter coupling) and the end-of-kernel
                    # drain waits less on the sync path than on gpsimd.
                    for m in range(MO):
                        otm = ob.tile([P, NSH], mybir.dt.bfloat16, tag="olast")
                        nc.vector.tensor_copy(otm[:], pt[:, m, :])
                        nc.sync.dma_start(out_d[i, :, m, :], otm[:])
                else:
                    ot = ob.tile([P, MO, NSH], mybir.dt.bfloat16, tag="o")
                    nc.vector.tensor_copy(ot[:], pt[:])
                    nc.gpsimd.dma_start(out_d[i], ot[:])

    nc.compile()
    return nc


def kernel(x, cat_ids, W, b):
    global LAST_RESULTS
    from concourse import bass_utils

    x = np.asarray(x, dtype=np.float32)
    cat_ids_np = np.asarray(cat_ids).astype(np.int64)
    W = np.asarray(W, dtype=np.float32)
    b = np.asarray(b, dtype=np.float32)
    cats = [int(c) for c in cat_ids_np]

    # Same category-grouped order the program bakes in.
    used = []
    for c in cats:
        if c not in used:
            used.append(c)
    proc_order = [j for c in used for j in range(B) if cats[j] == c]

    # x: [B,S,DIN] -> [B,P(q),KO,S] bf16, in processed order, paired.
    xp = np.ascontiguousarray(
        x.reshape(B, S, KO, P).transpose(0, 3, 2, 1)
    ).astype(ml_dtypes.bfloat16)[proc_order]
    xp = np.ascontiguousarray(
        xp.reshape(B // 2, 2, P, KO, S).transpose(0, 2, 1, 3, 4)
    )

    in_maps = []
    for core in range(NCORES):
        Wc = W[:, :, core * NSH:(core + 1) * NSH]
        Wp = np.ascontiguousarray(
            Wc.reshape(C, KO, P, NSH).transpose(0, 2, 1, 3)
        ).astype(ml_dtypes.bfloat16)
        in_maps.append({"x": xp, "w": Wp})

    nc = _build_program(cats)
    res = bass_utils.run_bass_kernel_spmd(
        nc, in_maps, core_ids=list(range(NCORES))
    )
    LAST_RESULTS = res

    inv = np.argsort(np.asarray(proc_order))
    out = np.empty((B, S, DH), dtype=np.float32)
    for core in range(NCORES):
        oc = res.results[core]["out"]                    # [B(proc), P, MO, NSH] bf16
        # out[proc_order[i], m*128+p, n] = oc[i, p, m, n]
        oc = oc.reshape(B, P, MO, NSH).transpose(0, 2, 1, 3).reshape(B, S, NSH)
        out[:, :, core * NSH:(core + 1) * NSH] = oc[inv].astype(np.float32)

    if b.any():
        out += b[cats][:, None, :]
    return out


# revision 18
# speedup vs baseline: 1.0042x; 1.0042x over previous
"""Category-specific linear on 8 trn2 cores — hidden-dim sharding, resident W.

v3 changes vs v2 (486us baseline):
- Output in bf16 (halves out DMA 67->33.5MB/core; total HBM 151->117MB
  removes the ~90% HBM saturation that caused mid-kernel W-prefetch stalls).
  Host upcasts to fp32.
- Warmup matmuls are dep-free (no memzero) and issued first, so the PE
  starts at ~0.3us instead of ~9us and the HAM clock-gate flips during the
  unavoidable first-x DMA wait.
- First x pair loaded as two per-sample DMAs interleaved with the first
  group's per-ko W pieces, so the k-pipeline of sample 0 never starves.
- W prefetch lookahead 3 groups (was 2), each as two half-tile DMAs so
  first-half matmuls can start before the full megabyte lands.
- Last sample drains PSUM per-m (4 small copies+DMAs) to shorten the tail.
"""

import numpy as np
import ml_dtypes

B = 64
S = 512
DIN = 1024
DH = 4096
C = 16
NCORES = 8
NSH = DH // NCORES   # 512
P = 128
KO = DIN // P        # 8
MO = S // P          # 4

LAST_RESULTS = None


def _build_program(cats):
    import concourse.bacc as bacc
    import concourse.mybir as mybir
    import concourse.tile as tile

    nc = bacc.Bacc("TRN2", target_bir_lowering=False)

    assert B % 2 == 0
    x_d = nc.dram_tensor("x", (B // 2, P, 2, KO, S), mybir.dt.bfloat16,
                         kind="ExternalInput")
    w_d = nc.dram_tensor("w", (C, P, KO, NSH), mybir.dt.bfloat16,
                         kind="ExternalInput")
    out_d = nc.dram_tensor("out", (B, P, MO, NSH), mybir.dt.bfloat16,
                           kind="ExternalOutput")

    # Category-grouped processing order: samples of the same category are
    # consecutive, so each category's W slice is prefetched once and covered
    # by ~n_c * 6.9us of compute. proc_order[i] = original sample index.
    used = []
    for c in cats:
        if c not in used:
            used.append(c)
    proc_order = [j for c in used for j in range(B) if cats[j] == c]
    # group g -> category used[g]; first processed position of each group
    group_start = {}
    for i, j in enumerate(proc_order):
        group_start.setdefault(cats[j], i)
    n_groups = len(used)

    with tile.TileContext(nc) as tc:
        with (
            tc.tile_pool(name="wpool", bufs=1) as wp,
            tc.tile_pool(name="xpool", bufs=3) as xb,
            tc.tile_pool(name="opool", bufs=3) as ob,
            tc.tile_pool(name="psum", bufs=2, space="PSUM") as ps,
        ):
            w_tiles = {}

            def emit_w(g):
                if g >= n_groups:
                    return
                c = used[g]
                if c in w_tiles:
                    return
                t = wp.tile([P, KO, NSH], mybir.dt.bfloat16, tag=f"w{c}")
                # two half-tile DMAs: matmuls on ko<4 only wait for half 0
                nc.sync.dma_start(t[:, 0:KO // 2, :], w_d[c, :, 0:KO // 2, :])
                nc.sync.dma_start(t[:, KO // 2:KO, :], w_d[c, :, KO // 2:KO, :])
                w_tiles[c] = t

            # PE warmup: matmuls on DVE-memset scratch, gated only on the two
            # fast memsets (~0.5us) rather than any DMA. They run during the
            # initial DMA wait so the HAM clock-gate reaches 8/8 before real
            # data lands.
            warm_l = wp.tile([P, P], mybir.dt.bfloat16, tag="warm_l")
            warm_r = wp.tile([P, NSH], mybir.dt.bfloat16, tag="warm_r")
            nc.vector.memset(warm_l[:], 0.0)
            nc.vector.memset(warm_r[:], 0.0)
            warm_p = ps.tile([P, MO, NSH], mybir.dt.float32, tag="ps")
            for _ in range(8):
                nc.tensor.matmul(warm_p[:, 0, :], warm_l[:], warm_r[:],
                                 start=True, stop=True)

            # Preamble DMAs in DEADLINE order. Each sync.dma_start costs
            # ~0.65us of issue time on the sync sequencer and the transfers
            # queue up behind each other at ~2.8us/MiB, so issue order must
            # match the order the data is consumed: otherwise the second x
            # pair queues behind 4MB of W prefetch and the PE stalls ~5us at
            # t~14us (observed in v3), re-throttling the HAM clock gate.
            c0 = used[0]
            w0k = [None] * KO
            xt_pairs = {}

            def emit_xpair(p):
                t = xb.tile([P, 2, KO, S], mybir.dt.bfloat16, tag="x")
                nc.sync.dma_start(t[:], x_d[p])
                xt_pairs[p] = t

            def emit_w0(ks):
                for k in ks:
                    twk = wp.tile([P, NSH], mybir.dt.bfloat16, tag=f"w{c0}_k{k}")
                    nc.sync.dma_start(twk[:], w_d[c0, :, k, :])
                    w0k[k] = twk

            # Pairs 1-2 are pre-issued beyond pair 0 (exactly filling the 3
            # x slots). More is counterproductive: DMA-completion waits are
            # thresholds on shared queue counters, so pre-queueing transfers
            # that are needed LATER inflates the wait of everything behind
            # them in the queue (measured: a 6-pair preamble stalled the PE
            # ~7us at t~59us waiting for a counter that included xp4/xp5).
            # Pair 2 in-loop was a razor-edge race: it lost by ~5.5us in one
            # run (16-warmup variant) and won by <1us in another.
            T_MM = 6.9  # us of matmul per sample
            PRE_PAIRS = min(3, B // 2)
            xt0 = xb.tile([P, 2, KO, S], mybir.dt.bfloat16, tag="x")
            xt_pairs[0] = xt0
            H = KO // 2
            pre = [(-3.0, "xs0a"), (-2.0, "w0k03"), (-1.0, "xs0b"),
                   (0.0, "w0k47"), (1.0, "xs1")]
            for p in range(1, PRE_PAIRS):
                pre.append((4 + 2 * T_MM * p - 6.0, f"xp{p}"))
            for g in range(1, min(4, n_groups)):
                pre.append((4 + T_MM * group_start[used[g]] - 8.0, f"wg{g}"))
            pre.sort()
            for _, what in pre:
                if what == "xs0a":
                    nc.sync.dma_start(xt0[:, 0, 0:H], x_d[0, :, 0, 0:H])
                elif what == "xs0b":
                    nc.sync.dma_start(xt0[:, 0, H:KO], x_d[0, :, 0, H:KO])
                elif what == "xs1":
                    nc.sync.dma_start(xt0[:, 1], x_d[0, :, 1])
                elif what == "w0k03":
                    emit_w0(range(0, 4))
                elif what == "w0k47":
                    emit_w0(range(4, KO))
                elif what.startswith("xp"):
                    emit_xpair(int(what[2:]))
                else:
                    emit_w(int(what[2:]))

            for i in range(B):
                j = proc_order[i]
                c = cats[j]
                g = used.index(c)
                if group_start[c] == i and g + 3 < n_groups:
                    emit_w(g + 3)
                if i % 2 == 0:
                    p = i // 2
                    if p in xt_pairs:
                        xt = xt_pairs.pop(p)
                    else:
                        xt = xb.tile([P, 2, KO, S], mybir.dt.bfloat16, tag="x")
                        nc.sync.dma_start(xt[:], x_d[p])
                pt = ps.tile([P, MO, NSH], mybir.dt.float32, tag="ps")
                for m in range(MO):
                    for k in range(KO):
                        rhs = w0k[k][:] if c == c0 else w_tiles[c][:, k, :]
                        nc.tensor.matmul(
                            pt[:, m, :],
                            xt[:, i % 2, k, m * P:(m + 1) * P],
                            rhs,
                            start=(k == 0),
                            stop=(k == KO - 1),
                        )
                # out DMAs are issued from GpSimd, NOT sync: DMA-completion
                # waits are thresholds on a per-queue counter, so if out
                # writes share a queue with x/W reads, every later x consumer
                # transitively waits on cast-paced out completions (measured
                # as a one-time ~7us PE stall at the first in-loop x pair).
                if i == B - 1:
                    # per-m drain: copy+store each 128x512 slab as soon as its
                    # k-accumulation finishes, shortening the kernel tail.
                    # These last stores go back on sync: no x/W reads remain
                    # after them (no counter coupling) and the end-of-kernel
                    # drain waits less on the sync path than on gpsimd.
                    for m in range(MO):
                        otm = ob.tile([P, NSH], mybir.dt.bfloat16, tag="olast")
                        nc.vector.tensor_copy(otm[:], pt[:, m, :])
                        nc.sync.dma_start(out_d[i, :, m, :], otm[:])
                else:
                    ot = ob.tile([P, MO, NSH], mybir.dt.bfloat16, tag="o")
                    nc.vector.tensor_copy(ot[:], pt[:])
                    nc.gpsimd.dma_start(out_d[i], ot[:])

    nc.compile()
    return nc


def kernel(x, cat_ids, W, b):
    global LAST_RESULTS
    from concourse import bass_utils

    x = np.asarray(x, dtype=np.float32)
    cat_ids_np = np.asarray(cat_ids).astype(np.int64)
    W = np.asarray(W, dtype=np.float32)
    b = np.asarray(b, dtype=np.float32)
    cats = [int(c) for c in cat_ids_np]

    # Same category-grouped order the program bakes in.
    used = []
    for c in cats:
        if c not in used:
            used.append(c)
    proc_order = [j for c in used for j in range(B) if cats[j] == c]

    # x: [B,S,DIN] -> [B,P(q),KO,S] bf16, in processed order, paired.
    xp = np.ascontiguousarray(
        x.reshape(B, S, KO, P).transpose(0, 3, 2, 1)
    ).astype(ml_dtypes.bfloat16)[proc_order]
    xp = np.ascontiguousarray(
        xp.reshape(B // 2, 2, P, KO, S).transpose(0, 2, 1, 3, 4)
    )

    in_maps = []
    for core in range(NCORES):
        Wc = W[:, :, core * NSH:(core + 1) * NSH]
        Wp = np.ascontiguousarray(
            Wc.reshape(C, KO, P, NSH).transpose(0, 2, 1, 3)
        ).astype(ml_dtypes.bfloat16)
        in_maps.append({"x": xp, "w": Wp})

    nc = _build_program(cats)
    res = bass_utils.run_bass_kernel_spmd(
        nc, in_maps, core_ids=list(range(NCORES))
    )
    LAST_RESULTS = res

    inv = np.argsort(np.asarray(proc_order))
    out = np.empty((B, S, DH), dtype=np.float32)
    for core in range(NCORES):
        oc = res.results[core]["out"]                    # [B(proc), P, MO, NSH] bf16
        # out[proc_order[i], m*128+p, n] = oc[i, p, m, n]
        oc = oc.reshape(B, P, MO, NSH).transpose(0, 2, 1, 3).reshape(B, S, NSH)
        out[:, :, core * NSH:(core + 1) * NSH] = oc[inv].astype(np.float32)

    if b.any():
        out += b[cats][:, None, :]
    return out


# revision 21
# speedup vs baseline: 1.0113x; 1.0070x over previous
"""Category-specific linear on 8 trn2 cores — hidden-dim sharding, resident W.

v3 changes vs v2 (486us baseline):
- Output in bf16 (halves out DMA 67->33.5MB/core; total HBM 151->117MB
  removes the ~90% HBM saturation that caused mid-kernel W-prefetch stalls).
  Host upcasts to fp32.
- Warmup matmuls are dep-free (no memzero) and issued first, so the PE
  starts at ~0.3us instead of ~9us and the HAM clock-gate flips during the
  unavoidable first-x DMA wait.
- First x pair loaded as two per-sample DMAs interleaved with the first
  group's per-ko W pieces, so the k-pipeline of sample 0 never starves.
- W prefetch lookahead 3 groups (was 2), each as two half-tile DMAs so
  first-half matmuls can start before the full megabyte lands.
- Last sample drains PSUM per-m (4 small copies+DMAs) to shorten the tail.
"""

import numpy as np
import ml_dtypes

B = 64
S = 512
DIN = 1024
DH = 4096
C = 16
NCORES = 8
NSH = DH // NCORES   # 512
P = 128
KO = DIN // P        # 8
MO = S // P          # 4

LAST_RESULTS = None


def _build_program(cats):
    import concourse.bacc as bacc
    import concourse.mybir as mybir
    import concourse.tile as tile

    nc = bacc.Bacc("TRN2", target_bir_lowering=False)

    assert B % 2 == 0
    x_d = nc.dram_tensor("x", (B // 2, P, 2, KO, S), mybir.dt.bfloat16,
                         kind="ExternalInput")
    w_d = nc.dram_tensor("w", (C, P, KO, NSH), mybir.dt.bfloat16,
                         kind="ExternalInput")
    out_d = nc.dram_tensor("out", (B, P, MO, NSH), mybir.dt.bfloat16,
                           kind="ExternalOutput")

    # Category-grouped processing order: samples of the same category are
    # consecutive, so each category's W slice is prefetched once and covered
    # by ~n_c * 6.9us of compute. proc_order[i] = original sample index.
    used = []
    for c in cats:
        if c not in used:
            used.append(c)
    proc_order = [j for c in used for j in range(B) if cats[j] == c]
    # group g -> category used[g]; first processed position of each group
    group_start = {}
    for i, j in enumerate(proc_order):
        group_start.setdefault(cats[j], i)
    n_groups = len(used)

    with tile.TileContext(nc) as tc:
        with (
            tc.tile_pool(name="wpool", bufs=1) as wp,
            tc.tile_pool(name="xpool", bufs=3) as xb,
            tc.tile_pool(name="opool", bufs=3) as ob,
            tc.tile_pool(name="psum", bufs=2, space="PSUM") as ps,
        ):
            w_tiles = {}

            def emit_w(g):
                if g >= n_groups:
                    return
                c = used[g]
                if c in w_tiles:
                    return
                t = wp.tile([P, KO, NSH], mybir.dt.bfloat16, tag=f"w{c}")
                # two half-tile DMAs: matmuls on ko<4 only wait for half 0
                nc.sync.dma_start(t[:, 0:KO // 2, :], w_d[c, :, 0:KO // 2, :])
                nc.sync.dma_start(t[:, KO // 2:KO, :], w_d[c, :, KO // 2:KO, :])
                w_tiles[c] = t

            # PE warmup: matmuls on DVE-memset scratch, gated only on the two
            # fast memsets (~0.5us) rather than any DMA. They run during the
            # initial DMA wait so the HAM clock-gate reaches 8/8 before real
            # data lands.
            warm_l = wp.tile([P, P], mybir.dt.bfloat16, tag="warm_l")
            warm_r = wp.tile([P, NSH], mybir.dt.bfloat16, tag="warm_r")
            nc.vector.memset(warm_l[:], 0.0)
            nc.vector.memset(warm_r[:], 0.0)
            warm_p = ps.tile([P, MO, NSH], mybir.dt.float32, tag="ps")
            # 16 warmups = ~6.8us cold-rate PE busy from the ~7us engine-boot
            # point: the HAM clock-gate flips at ~10.5us and the first-sample
            # x/W pieces (which trickle in until ~13us and caused 0.1-3.2us
            # late-arrival stutters when real MMs started at ~10.5) are all
            # resident before the first real matmul at ~13.8us.
            for _ in range(16):
                nc.tensor.matmul(warm_p[:, 0, :], warm_l[:], warm_r[:],
                                 start=True, stop=True)

            # Preamble DMAs in DEADLINE order. Each sync.dma_start costs
            # ~0.65us of issue time on the sync sequencer and the transfers
            # queue up behind each other at ~2.8us/MiB, so issue order must
            # match the order the data is consumed: otherwise the second x
            # pair queues behind 4MB of W prefetch and the PE stalls ~5us at
            # t~14us (observed in v3), re-throttling the HAM clock gate.
            c0 = used[0]
            w0k = [None] * KO
            xt_pairs = {}

            def emit_xpair(p):
                t = xb.tile([P, 2, KO, S], mybir.dt.bfloat16, tag="x")
                nc.sync.dma_start(t[:], x_d[p])
                xt_pairs[p] = t

            def emit_w0(ks):
                for k in ks:
                    twk = wp.tile([P, NSH], mybir.dt.bfloat16, tag=f"w{c0}_k{k}")
                    nc.sync.dma_start(twk[:], w_d[c0, :, k, :])
                    w0k[k] = twk

            # Pairs 1-2 pre-issued beyond pair 0 (fills the 3 x slots; pair 2
            # in-loop is a razor-edge race that cost 5.5us when lost). More is
            # counterproductive: DMA-completion waits are thresholds on shared
            # queue counters, so pre-queueing transfers needed LATER inflates
            # the wait of everything behind them (a 6-pair preamble stalled
            # the PE ~7us waiting on a counter that included xp4/xp5).
            T_MM = 6.9  # us of matmul per sample
            PRE_PAIRS = min(3, B // 2)
            xt0 = xb.tile([P, 2, KO, S], mybir.dt.bfloat16, tag="x")
            xt_pairs[0] = xt0
            H = KO // 2
            pre = [(-3.0, "xs0a"), (-2.0, "w0k03"), (-1.0, "xs0b"),
                   (0.0, "w0k47"), (1.0, "xs1")]
            for p in range(1, PRE_PAIRS):
                pre.append((4 + 2 * T_MM * p - 6.0, f"xp{p}"))
            for g in range(1, min(4, n_groups)):
                pre.append((4 + T_MM * group_start[used[g]] - 8.0, f"wg{g}"))
            pre.sort()
            for _, what in pre:
                if what == "xs0a":
                    nc.sync.dma_start(xt0[:, 0, 0:H], x_d[0, :, 0, 0:H])
                elif what == "xs0b":
                    nc.sync.dma_start(xt0[:, 0, H:KO], x_d[0, :, 0, H:KO])
                elif what == "xs1":
                    nc.sync.dma_start(xt0[:, 1], x_d[0, :, 1])
                elif what == "w0k03":
                    emit_w0(range(0, 4))
                elif what == "w0k47":
                    emit_w0(range(4, KO))
                elif what.startswith("xp"):
                    emit_xpair(int(what[2:]))
                else:
                    emit_w(int(what[2:]))

            for i in range(B):
                j = proc_order[i]
                c = cats[j]
                g = used.index(c)
                if group_start[c] == i and g + 3 < n_groups:
                    emit_w(g + 3)
                if i % 2 == 0:
                    p = i // 2
                    if p in xt_pairs:
                        xt = xt_pairs.pop(p)
                    else:
                        xt = xb.tile([P, 2, KO, S], mybir.dt.bfloat16, tag="x")
                        nc.sync.dma_start(xt[:], x_d[p])
                pt = ps.tile([P, MO, NSH], mybir.dt.float32, tag="ps")
                for m in range(MO):
                    for k in range(KO):
                        rhs = w0k[k][:] if c == c0 else w_tiles[c][:, k, :]
                        nc.tensor.matmul(
                            pt[:, m, :],
                            xt[:, i % 2, k, m * P:(m + 1) * P],
                            rhs,
                            start=(k == 0),
                            stop=(k == KO - 1),
                        )
                # out DMAs are issued from GpSimd, NOT sync: DMA-completion
                # waits are thresholds on a per-queue counter, so if out
                # writes share a queue with x/W reads, every later x consumer
                # transitively waits on cast-paced out completions (measured
                # as a one-time ~7us PE stall at the first in-loop x pair).
                if i == B - 1:
                    # per-m drain: copy+store each 128x512 slab as soon as its
                    # k-accumulation finishes, shortening the kernel tail.
                    # These last stores go back on sync: no x/W reads remain
                    # after them (no counter coupling) and the end-of-kernel
                    # drain waits less on the sync path than on gpsimd.
                    for m in range(MO):
                        otm = ob.tile([P, NSH], mybir.dt.bfloat16, tag="olast")
                        nc.vector.tensor_copy(otm[:], pt[:, m, :])
                        nc.sync.dma_start(out_d[i, :, m, :], otm[:])
                else:
                    ot = ob.tile([P, MO, NSH], mybir.dt.bfloat16, tag="o")
                    nc.vector.tensor_copy(ot[:], pt[:])
                    nc.gpsimd.dma_start(out_d[i], ot[:])

    nc.compile()
    return nc


def kernel(x, cat_ids, W, b):
    global LAST_RESULTS
    from concourse import bass_utils

    x = np.asarray(x, dtype=np.float32)
    cat_ids_np = np.asarray(cat_ids).astype(np.int64)
    W = np.asarray(W, dtype=np.float32)
    b = np.asarray(b, dtype=np.float32)
    cats = [int(c) for c in cat_ids_np]

    # Same category-grouped order the program bakes in.
    used = []
    for c in cats:
        if c not in used:
            used.append(c)
    proc_order = [j for c in used for j in range(B) if cats[j] == c]

    # x: [B,S,DIN] -> [B,P(q),KO,S] bf16, in processed order, paired.
    xp = np.ascontiguousarray(
        x.reshape(B, S, KO, P).transpose(0, 3, 2, 1)
    ).astype(ml_dtypes.bfloat16)[proc_order]
    xp = np.ascontiguousarray(
        xp.reshape(B // 2, 2, P, KO, S).transpose(0, 2, 1, 3, 4)
    )

    in_maps = []
    for core in range(NCORES):
        Wc = W[:, :, core * NSH:(core + 1) * NSH]
        Wp = np.ascontiguousarray(
            Wc.reshape(C, KO, P, NSH).transpose(0, 2, 1, 3)
        ).astype(ml_dtypes.bfloat16)
        in_maps.append({"x": xp, "w": Wp})

    nc = _build_program(cats)
    res = bass_utils.run_bass_kernel_spmd(
        nc, in_maps, core_ids=list(range(NCORES))
    )
    LAST_RESULTS = res

    inv = np.argsort(np.asarray(proc_order))
    out = np.empty((B, S, DH), dtype=np.float32)
    for core in range(NCORES):
        oc = res.results[core]["out"]                    # [B(proc), P, MO, NSH] bf16
        # out[proc_order[i], m*128+p, n] = oc[i, p, m, n]
        oc = oc.reshape(B, P, MO, NSH).transpose(0, 2, 1, 3).reshape(B, S, NSH)
        out[:, :, core * NSH:(core + 1) * NSH] = oc[inv].astype(np.float32)

    if b.any():
        out += b[cats][:, None, :]
    return out
